# revision 23
# baseline (speedup 1.0000x reference)
"""Trainium2 Bass kernel for nn_Net_64982855188859 (ECC graph-conv net).

Network (per graph g of B=8, N=128 nodes, F=16 feats, S=8 edge feats, C=32):
  mask = x[..., -1:], h = x[..., :16]
  h = leaky_relu(ECC1(h, a, e) * mask)          ECC: per-edge MLP -> filters
  h = relu(ECC2(h, a, e)) * mask
  out = softmax(relu(mean_nodes(h) @ dw) @ ow)

Restructuring (all exact):
 1. w1c/w2c factor through the node features: xW[j,(k,c)] = sum_f x[j,f]
    w1c[k, f*C+c], so out[i,c] = sum_{j,k} a[i,j] h2[(j,i),k] xW[j,(k,c)].
 2. a >= 0 commutes through the zero-bias per-edge ReLU MLP, so the host
    pre-multiplies e by a (eA) and no masking happens on device.
 3. Both layers' per-edge MLPs depend only on eA, so they are FUSED:
    h1 = eA @ [w1a|w2a] (M=128), h2 = relu(h1) @ blockdiag(w1b,w2b)
    (M=64, K=128), halving PE streaming vs per-layer passes.
 4. h2 psum partition layout (j2, layer, k) via tile_position col packing:
    one [128, 8192] tensor h2B holds both layers' per-edge features in
    contraction-chunk layout (chunk = j-pair, K=128 with the other layer's
    rhs rows zeroed).
 5. Contraction is flipped: lhsT = h2B chunk [128,128] (FWL-fast weight
    loads), rhs = 32 cols of xwr -> out[i, c] in psum. Layer-1 xwr is
    HOST-computed (x is an input); layer-2 ywr needs one DRAM roundtrip.
 6. Per-core work = one graph (data-parallel over B=8 across 8 cores).
"""

import numpy as np

import concourse.bass as bass
import concourse.bacc as bacc
import concourse.mybir as mybir
import concourse.tile as tile
from concourse.bass_utils import run_bass_kernel_spmd

F32 = mybir.dt.float32
BF16 = mybir.dt.bfloat16
AF = mybir.ActivationFunctionType
ALU = mybir.AluOpType

B, N, F, S, C = 8, 128, 16, 8, 32
E = N * N               # 16384 edges per graph

# wbf column layout ([128, 256] bf16, shared across cores)
W1AB = slice(0, 128)      # rows 0:8  = [w1a | w2a]
WB = slice(128, 192)      # [128,64] blockdiag: rows 0:64 cols 0:32 = w1b,
                          #                     rows 64:128 cols 32:64 = w2b
ROOT2B = slice(192, 224)  # rows 0:32 = root2 (bf16)
SEL4 = slice(224, 256)    # sel4[p, c] = (p % 32 == c): 4-group psum reducer
WBF_COLS = 256
# wp column layout ([128, 192] fp32, shared)
ROOT1 = slice(0, 32)      # rows 0:16
DW = slice(32, 96)        # rows 0:32
OW = slice(96, 106)       # rows 0:64
DB = slice(106, 107)      # rows 0:64 (column vector)
BIAS1 = slice(112, 144)   # row 0
BIAS2 = slice(144, 176)   # row 0
OB = slice(176, 186)      # row 0
WP_COLS = 192


def build_nc(loop_n: int | None = None, unroll: bool = False,
             hoist: bool = True, skip: tuple = (), interleave_c1: bool = False):
    nc = bacc.Bacc("TRN2", target_bir_lowering=False, debug=False)
    eA_d = nc.dram_tensor("eA", [S, E], BF16, kind="ExternalInput").ap()
    xwr1_d = nc.dram_tensor("xwr1", [128, 2048], BF16, kind="ExternalInput").ap()
    xc_d = nc.dram_tensor("xc", [17, 128], F32, kind="ExternalInput").ap()
    wbf_d = nc.dram_tensor("wbf", [128, WBF_COLS], BF16, kind="ExternalInput").ap()
    wp_d = nc.dram_tensor("wp", [128, WP_COLS], F32, kind="ExternalInput").ap()
    w2p_d = nc.dram_tensor("w2p", [32, 1024], BF16, kind="ExternalInput").ap()
    out_d = nc.dram_tensor("out", [1, 10], F32, kind="ExternalOutput").ap()

    with tile.TileContext(nc) as tc:
        with (
            tc.tile_pool(name="consts", bufs=1) as consts,
            tc.tile_pool(name="eA", bufs=1) as pool_eA,
            tc.tile_pool(name="h1sp", bufs=2) as pool_h1s,
            tc.tile_pool(name="h2Bp", bufs=2) as pool_h2B,
            tc.tile_pool(name="xwr", bufs=1) as pool_xwr,
            tc.tile_pool(name="pmm", bufs=3, space="PSUM") as pool_pmm,
            tc.tile_pool(name="po", bufs=2, space="PSUM") as pool_po,
            tc.tile_pool(name="misc", bufs=1) as pool_misc,
            tc.tile_pool(name="dram", bufs=1, space="DRAM") as dram_pool,
        ):
            def setup():
                # ---- loop-invariant: weights + the xwr zero regions ----
                wbf_sb = consts.tile([128, WBF_COLS], BF16, tag="wbf")
                nc.sync.dma_start(out=wbf_sb[:, :], in_=wbf_d)
                wp_sb = consts.tile([128, WP_COLS], F32, tag="wp")
                nc.sync.dma_start(out=wp_sb[:, :], in_=wp_d)
                w2p_sb = consts.tile([32, 1024], BF16, tag="w2p")
                nc.sync.dma_start(out=w2p_sb[:, :], in_=w2p_d)
                ones_sb = consts.tile([1, 128], F32, tag="ones")
                nc.vector.memset(ones_sb[:, :], 1.0)
                # xwr [128, 4096]: cols 0:2048 layer-1 (rows 0:32/64:96 loaded
                # per-iteration), cols 2048:4096 layer-2 (l2 rows written by
                # the device restructure). All other rows must read as zero.
                xwr_sb = pool_xwr.tile([128, 4096], BF16, tag="xwr")
                nc.vector.memset(xwr_sb[:, :], 0.0)
                return (wbf_sb, wp_sb, w2p_sb, ones_sb, xwr_sb)

            def tail(state, h2B, y1bf, xwr_sb, oc2):
                (wbf_sb, wp_sb, w2p_sb, ones_sb, _xwr) = state
                # ---- contraction layer 2: 4 accumulate chains, one per PE
                # column tile; partials reduced by the sel4 matmul ----
                nc.tensor.matmul(out=oc2[0:32, :], lhsT=wp_sb[0:1, BIAS2],
                                 rhs=ones_sb[:, :],
                                 start=True, stop=False, skip_group_check=True)
                nc.tensor.matmul(out=oc2[32:64, :], lhsT=wbf_sb[0:32, ROOT2B],
                                 rhs=y1bf[:, :],
                                 start=True, stop=False, skip_group_check=True)
                for p in range(0 if "c2" in skip else 64):
                    q = 32 * (p % 4)
                    nc.tensor.matmul(
                        out=oc2[q:q + 32, :],
                        lhsT=xwr_sb[:, 2048 + 32 * p:2048 + 32 * p + 32],
                        rhs=h2B[:, 128 * p:128 * p + 128],
                        start=(2 <= p < 4), stop=(p >= 60),
                        skip_group_check=True, tile_position=(0, q),
                    )

                # reduce the 4 partial groups: evac + sel4 matmul
                oce2 = pool_misc.tile([128, 128], BF16, tag="oce2")
                nc.scalar.activation(out=oce2[:, :], in_=oc2[:, :],
                                     func=AF.Copy)
                o2f = pool_po.tile([C, 128], F32, tag="po", name="o2f")
                nc.tensor.matmul(out=o2f[:, :], lhsT=wbf_sb[0:128, SEL4],
                                 rhs=oce2[:, :], start=True, stop=True,
                                 skip_group_check=True)
                gv_sb = pool_misc.tile([C, 1], F32, tag="gv")
                h2f_sb = pool_misc.tile([C, 128], F32, tag="h2f")
                nc.scalar.activation(out=h2f_sb[:, :], in_=o2f[:, :],
                                     func=AF.Relu, accum_out=gv_sb[:, :])

                # ---- head: dense(64, relu)/128 -> dense(10) -> softmax ----
                d_ps = pool_po.tile([64, 1], F32, tag="po", name="d")
                nc.tensor.matmul(out=d_ps[:, :], lhsT=wp_sb[0:32, DW],
                                 rhs=gv_sb[:, :], start=True, stop=True)
                d_sb = pool_misc.tile([64, 1], F32, tag="d")
                nc.scalar.activation(out=d_sb[:, :], in_=d_ps[:, :], func=AF.Relu,
                                     bias=wp_sb[0:64, DB], scale=1.0 / 128.0)
                lg_ps = pool_po.tile([1, 10], F32, tag="po", name="lg")
                nc.tensor.matmul(out=lg_ps[:, :], lhsT=d_sb[:, :],
                                 rhs=wp_sb[0:64, OW], start=True, stop=True)
                lg_sb = pool_misc.tile([1, 10], F32, tag="lg")
                nc.vector.tensor_copy(lg_sb[:, :], lg_ps[:, :])
                nc.sync.dma_start(out=out_d, in_=lg_sb[:, :])

            def body(state, pipelined=False):
                (wbf_sb, wp_sb, w2p_sb, ones_sb, xwr_sb) = state
                # eA loads chunked (512 cols) so each h1 matmul only
                # waits for its own slice
                eA_sb = pool_eA.tile([S, E], BF16, tag="eA")
                if "eadma" not in skip:
                    for r in range(4):
                        nc.sync.dma_start(
                            out=eA_sb[:, 4096 * r:4096 * (r + 1)],
                            in_=eA_d[:, 4096 * r:4096 * (r + 1)])
                xc_sb = consts.tile([17, 128], F32, tag="xc")
                nc.sync.dma_start(out=xc_sb[:, :], in_=xc_d)

                nc.sync.dma_start(out=xwr_sb[:, 0:2048], in_=xwr1_d)

                h1s = pool_h1s.tile([128, E], BF16, tag="h1s")
                h2B = pool_h2B.tile([128, 8192], BF16, tag="h2B")
                y1bf = pool_misc.tile([C, 128], BF16, tag="y1bf")
                # contraction psums: 4 accumulate chains per layer, one per
                # PE column tile, reduced by the sel4 matmul at the end
                oc = pool_po.tile([128, 128], F32, tag="po", name="oc")
                oc2 = pool_po.tile([128, 128], F32, tag="po", name="oc2")

                nc.tensor.matmul(out=oc[0:32, :], lhsT=wp_sb[0:1, BIAS1],
                                 rhs=ones_sb[:, :],
                                 start=True, stop=False, skip_group_check=True)

                # ---- h1 (both layers fused, M=128, K=8) ----
                for r in range(0 if "h1" in skip else 16):
                    ps = pool_pmm.tile([128, 1024], F32, tag="pmm", name=f"h1_{r}")
                    for hb in range(2):
                        nc.tensor.matmul(
                            out=ps[:, 512 * hb:512 * (hb + 1)],
                            lhsT=wbf_sb[0:8, W1AB],
                            rhs=eA_sb[:, 1024 * r + 512 * hb:
                                      1024 * r + 512 * (hb + 1)],
                            start=True, stop=True,
                        )
                    dst = h1s[:, 1024 * r:1024 * (r + 1)]
                    if r % 2 == 1:
                        nc.vector.tensor_scalar_max(dst, ps[:, :], 0.0)
                    else:
                        nc.scalar.activation(out=dst, in_=ps[:, :], func=AF.Relu)

                if pipelined:
                    # previous iteration's tail, placed after h1 so its wait
                    # on the prev yw roundtrip hides behind h1's matmuls
                    tail(state, h2B, y1bf, xwr_sb, oc2)

                # ---- h2 (both layers fused, M=64, K=128, j-pair packing) ----
                # group g: 16 edge-js (8 pairs). Host orders eA columns as
                # (g, j2, pp, i) so each (j2, bank) matmul streams a
                # CONTIGUOUS 512-col run of h1s (strided rhs is ~4x slower).
                for g in range(0 if "h2" in skip else 8):
                    ps = pool_pmm.tile([128, 1024], F32, tag="pmm", name=f"h2_{g}")
                    for j2 in range(2):
                        for hb in range(2):
                            lo = 2048 * g + 1024 * j2 + 512 * hb
                            nc.tensor.matmul(
                                out=ps[64 * j2:64 * j2 + 64,
                                       512 * hb:512 * (hb + 1)],
                                lhsT=wbf_sb[0:128, WB],
                                rhs=h1s[:, lo:lo + 512],
                                start=True, stop=True,
                                tile_position=(0, 64 * j2),
                            )
                    dst = h2B[:, 1024 * g:1024 * (g + 1)]
                    if g % 2 == 0:
                        nc.vector.tensor_scalar_max(dst, ps[:, :], 0.0)
                    else:
                        nc.scalar.activation(out=dst, in_=ps[:, :], func=AF.Relu)
                    if g == 0:
                        # group-1 chain seed (needs xc; placed here so the
                        # matmul sits in the busy stream, not at body top)
                        nc.tensor.matmul(
                            out=oc[32:64, :], lhsT=wp_sb[0:16, ROOT1],
                            rhs=xc_sb[0:16, :],
                            start=True, stop=False, skip_group_check=True)
                # clean single-mode phase: 4 accumulate chains across PE
                # column tiles (0,0)/(0,32)/(0,64)/(0,96)
                for p in range(0 if "c1" not in skip else 64, 64):
                    q = 32 * (p % 4)
                    nc.tensor.matmul(
                        out=oc[q:q + 32, :],
                        lhsT=xwr_sb[:, 32 * p:32 * p + 32],
                        rhs=h2B[:, 128 * p:128 * p + 128],
                        start=(2 <= p < 4), stop=(p >= 60),
                        skip_group_check=True, tile_position=(0, q),
                    )

                # reduce the 4 partial groups + leaky relu (mask is all-ones)
                oce = pool_misc.tile([128, 128], BF16, tag="oce")
                nc.scalar.activation(out=oce[:, :], in_=oc[:, :], func=AF.Copy)
                o1f = pool_po.tile([C, 128], F32, tag="po", name="o1f")
                nc.tensor.matmul(out=o1f[:, :], lhsT=wbf_sb[0:128, SEL4],
                                 rhs=oce[:, :], start=True, stop=True,
                                 skip_group_check=True)
                y1f = pool_misc.tile([C, 128], F32, tag="y1f")
                nc.scalar.activation(out=y1f[:, :], in_=o1f[:, :],
                                     func=AF.Copy)
                nc.vector.scalar_tensor_tensor(
                    out=y1bf[:, :], in0=y1f[:, :], scalar=0.05, in1=y1f[:, :],
                    op0=ALU.mult, op1=ALU.max)

                # ---- yW = y1 @ w2p, restructured into xwr cols 2048:4096 ----
                yw_ps = pool_pmm.tile([128, 1024], F32, tag="pmm", name="yw")
                for hb in range(2):
                    nc.tensor.matmul(out=yw_ps[:, 512 * hb:512 * (hb + 1)],
                                     lhsT=y1bf[:, :],
                                     rhs=w2p_sb[:, 512 * hb:512 * (hb + 1)],
                                     start=True, stop=True)
                yw_bf = pool_misc.tile([128, 1024], BF16, tag="ywbf")
                nc.scalar.activation(out=yw_bf[:, 0:512], in_=yw_ps[:, 0:512],
                                     func=AF.Copy)
                nc.vector.tensor_copy(yw_bf[:, 512:1024], yw_ps[:, 512:1024])
                ywd = dram_pool.tile([128, 1024], BF16, tag="ywd")
                ywr = ywd[:, :].rearrange("(p j2) (k c) -> j2 k p c", j2=2, c=32)
                nc.sync.dma_start(out=ywd[:, :], in_=yw_bf[:, :])
                for j2 in range(2):
                    nc.sync.dma_start(
                        out=xwr_sb[64 * j2 + 32:64 * j2 + 64, 2048:4096
                                   ].rearrange("k (p c) -> k p c", c=32),
                        in_=ywr[j2, :, :, :],
                    )

                if not pipelined:
                    tail(state, h2B, y1bf, xwr_sb, oc2)

            if loop_n is not None and loop_n > 1:
                if unroll:
                    st = setup()
                    for _ in range(loop_n):
                        body(st)
                elif hoist:
                    st = setup()
                    with tc.For_i(0, loop_n, 1, hint_engines=(
                            mybir.EngineType.PE, mybir.EngineType.DVE,
                            mybir.EngineType.Activation, mybir.EngineType.SP)):
                        body(st, pipelined=True)
                else:
                    with tc.For_i(0, loop_n, 1, hint_engines=(
                            mybir.EngineType.PE, mybir.EngineType.DVE,
                            mybir.EngineType.Activation, mybir.EngineType.SP)):
                        body(setup())
            else:
                body(setup())
    nc.compile()
    return nc


def prep_inputs(x, a, e, w1a, b1a, w1b, b1b, w1c, b1c, root1, bias1,
                w2a, b2a, w2b, b2b, w2c, b2c, root2, bias2, dw, db, ow, ob):
    """Host-side shard + layout prep. Returns in_maps (one per core)."""
    x = np.asarray(x, np.float32)
    a = np.asarray(a, np.float32)
    e = np.asarray(e, np.float32)
    # The device program folds a into e through the zero-bias ReLU MLP.
    for b_ in (b1a, b1b, b1c, b2a, b2b, b2c):
        assert np.abs(np.asarray(b_)).max() == 0.0, "nonzero MLP bias unsupported"
    # GraphMasking node-validity column is structurally all-ones here; the
    # device program relies on it (mask multiplies are elided).
    assert np.all(x[:, :, 16] == 1.0), "non-trivial node mask unsupported"

    import ml_dtypes
    bf16 = ml_dtypes.bfloat16

    wbf = np.zeros((128, WBF_COLS), bf16)
    wbf[0:8, 0:64] = np.asarray(w1a).astype(bf16)
    wbf[0:8, 64:128] = np.asarray(w2a).astype(bf16)
    wbf[0:64, 128:160] = np.asarray(w1b).astype(bf16)
    wbf[64:128, 160:192] = np.asarray(w2b).astype(bf16)
    wbf[0:32, ROOT2B] = np.asarray(root2).astype(bf16)
    sel4 = np.zeros((128, 32), np.float32)
    sel4[np.arange(128), np.arange(128) % 32] = 1.0
    wbf[:, SEL4] = sel4.astype(bf16)

    wp = np.zeros((128, WP_COLS), np.float32)
    wp[0:16, ROOT1] = np.asarray(root1)
    wp[0:32, DW] = np.asarray(dw)
    wp[0:64, OW] = np.asarray(ow)
    wp[0:64, DB] = np.asarray(db).reshape(64, 1)
    wp[0:1, BIAS1] = np.asarray(bias1).reshape(1, 32)
    wp[0:1, BIAS2] = np.asarray(bias2).reshape(1, 32)
    wp[0:1, OB] = np.asarray(ob).reshape(1, 10)

    # w1p[f, 32k+c] = w1c[k, f*32+c]; w2p[cin, 32k+c] = w2c[k, 32*cin+c]
    w1p = np.ascontiguousarray(
        np.asarray(w1c, np.float32).reshape(32, 16, 32)
        .transpose(1, 0, 2).reshape(16, 1024))
    w2p = np.ascontiguousarray(
        np.asarray(w2c).reshape(32, 32, 32).transpose(1, 0, 2).reshape(32, 1024)
    ).astype(bf16)

    in_maps = []
    for g in range(B):
        eA = np.ascontiguousarray(
            (e[g] * a[g][..., None]).transpose(2, 1, 0).reshape(S, E))
        # reorder edge columns j-major -> (group, j2=j%2, pp=(j%16)//2, i) so
        # the h2 matmuls stream contiguous runs
        eA = np.ascontiguousarray(
            eA.reshape(S, 8, 8, 2, 128).transpose(0, 1, 3, 2, 4).reshape(S, E)
        ).astype(bf16)
        x16 = x[g, :, 0:16]                     # [128, 16]
        xc = np.ascontiguousarray(x[g].T)       # [17, 128]
        # xwr1[32*j2 + k, 32*p + c] = xW[2p + j2, 32k + c]
        xW = (x16 @ w1p).astype(np.float32)     # [128, 1024]
        # merged layout [128, 2048]: rows 64*j2+0:32 = layer-1 blk(j2),
        # rows 64*j2+32:64 = zeros (layer-2 k slots read as 0 in cols 0:2048)
        xwr1 = np.zeros((128, 2048), np.float32)
        for j2 in range(2):
            blk = xW[j2::2].reshape(64, 32, 32).transpose(1, 0, 2).reshape(32, 2048)
            xwr1[64 * j2:64 * j2 + 32, :] = blk
        in_maps.append(dict(
            eA=eA, xwr1=xwr1.astype(bf16), xc=xc,
            wbf=wbf, wp=wp, w2p=w2p))
    return in_maps


_NC_CACHE = {}


def _get_nc(loop_n=None):
    key = loop_n
    if key not in _NC_CACHE:
        _NC_CACHE[key] = build_nc(loop_n)
    return _NC_CACHE[key]


def kernel(**inputs) -> np.ndarray:
    in_maps = prep_inputs(**inputs)
    nc = _get_nc()
    # The axon-tunneled device occasionally reports a transient
    # "exec unit unrecoverable" on the first dispatch after idle; retry.
    last = None
    for _ in range(3):
        try:
            res = run_bass_kernel_spmd(nc, in_maps, core_ids=list(range(B)))
            lg = np.concatenate(
                [res.results[g]["out"] for g in range(B)], axis=0)
            lg = lg.astype(np.float64) + np.asarray(inputs["ob"], np.float64)
            ex = np.exp(lg - lg.max(axis=1, keepdims=True))
            return (ex / ex.sum(axis=1, keepdims=True)).astype(np.float32)
        except Exception as ex:  # noqa: BLE001
            last = ex
    raise last



# revision 27
# speedup vs baseline: 1.4405x; 1.4405x over previous
"""Trainium2 Bass kernel for nn_Net_64982855188859 (ECC graph-conv net).

Network (per graph g of B=8, N=128 nodes, F=16 feats, S=8 edge feats, C=32):
  mask = x[..., -1:], h = x[..., :16]
  h = leaky_relu(ECC1(h, a, e) * mask)          ECC: per-edge MLP -> filters
  h = relu(ECC2(h, a, e)) * mask
  out = softmax(relu(mean_nodes(h) @ dw) @ ow)

Restructuring (all exact):
 1. w1c/w2c factor through the node features: xW[j,(k,c)] = sum_f x[j,f]
    w1c[k, f*C+c], so out[i,c] = sum_{j,k} a[i,j] h2[(j,i),k] xW[j,(k,c)].
 2. a >= 0 commutes through the zero-bias per-edge ReLU MLP, so the host
    pre-multiplies e by a (eA) and no masking happens on device.
 3. Both layers' per-edge MLPs depend only on eA, so they are FUSED:
    h1 = eA @ [w1a|w2a] (M=128), h2 = relu(h1) @ blockdiag(w1b,w2b)
    (M=64, K=128), halving PE streaming vs per-layer passes.
 4. h2 psum partition layout (j2, layer, k) via tile_position col packing:
    one [128, 8192] tensor h2B holds both layers' per-edge features in
    contraction-chunk layout (chunk = j-pair, K=128 with the other layer's
    rhs rows zeroed).
 5. Contraction is flipped: lhsT = h2B chunk [128,128] (FWL-fast weight
    loads), rhs = 32 cols of xwr -> out[i, c] in psum. Layer-1 xwr is
    HOST-computed (x is an input); layer-2 ywr needs one DRAM roundtrip.
 6. Per-core work = one graph (data-parallel over B=8 across 8 cores).
"""

import numpy as np

import concourse.bass as bass
import concourse.bacc as bacc
import concourse.mybir as mybir
import concourse.tile as tile
from concourse.bass_utils import run_bass_kernel_spmd

F32 = mybir.dt.float32
BF16 = mybir.dt.bfloat16
AF = mybir.ActivationFunctionType
ALU = mybir.AluOpType

B, N, F, S, C = 8, 128, 16, 8, 32
E = N * N               # 16384 edges per graph

# wbf column layout ([128, 256] bf16, shared across cores)
W1AB = slice(0, 128)      # rows 0:8  = [w1a | w2a]
WB = slice(128, 192)      # [128,64] blockdiag: rows 0:64 cols 0:32 = w1b,
                          #                     rows 64:128 cols 32:64 = w2b
ROOT2B = slice(192, 224)  # rows 0:32 = root2 (bf16)
SEL4 = slice(224, 256)    # sel4[p, c] = (p % 32 == c): 4-group psum reducer
WBF_COLS = 256
# wp column layout ([128, 192] fp32, shared)
ROOT1 = slice(0, 32)      # rows 0:16
DW = slice(32, 96)        # rows 0:32
OW = slice(96, 106)       # rows 0:64
DB = slice(106, 107)      # rows 0:64 (column vector)
BIAS1 = slice(112, 144)   # row 0
BIAS2 = slice(144, 176)   # row 0
OB = slice(176, 186)      # row 0
WP_COLS = 192


def build_nc(loop_n: int | None = None, unroll: bool = False,
             hoist: bool = True, skip: tuple = (), unroll_factor: int = 4):
    nc = bacc.Bacc("TRN2", target_bir_lowering=False, debug=False)
    eA_d = nc.dram_tensor("eA", [S, E], BF16, kind="ExternalInput").ap()
    xwr1_d = nc.dram_tensor("xwr1", [128, 2048], BF16, kind="ExternalInput").ap()
    xc_d = nc.dram_tensor("xc", [17, 128], F32, kind="ExternalInput").ap()
    wbf_d = nc.dram_tensor("wbf", [128, WBF_COLS], BF16, kind="ExternalInput").ap()
    wp_d = nc.dram_tensor("wp", [128, WP_COLS], F32, kind="ExternalInput").ap()
    w2p_d = nc.dram_tensor("w2p", [32, 1024], BF16, kind="ExternalInput").ap()
    out_d = nc.dram_tensor("out", [1, 10], F32, kind="ExternalOutput").ap()

    with tile.TileContext(nc) as tc:
        with (
            tc.tile_pool(name="consts", bufs=1) as consts,
            tc.tile_pool(name="eA", bufs=1) as pool_eA,
            tc.tile_pool(name="h1sp", bufs=2) as pool_h1s,
            tc.tile_pool(name="h2Bp", bufs=2) as pool_h2B,
            tc.tile_pool(name="xwr", bufs=1) as pool_xwr,
            tc.tile_pool(name="pmm", bufs=3, space="PSUM") as pool_pmm,
            tc.tile_pool(name="po", bufs=2, space="PSUM") as pool_po,
            tc.tile_pool(name="misc", bufs=1) as pool_misc,
            tc.tile_pool(name="dram", bufs=1, space="DRAM") as dram_pool,
        ):
            def setup():
                # ---- loop-invariant: weights + the xwr zero regions ----
                wbf_sb = consts.tile([128, WBF_COLS], BF16, tag="wbf")
                nc.sync.dma_start(out=wbf_sb[:, :], in_=wbf_d)
                wp_sb = consts.tile([128, WP_COLS], F32, tag="wp")
                nc.sync.dma_start(out=wp_sb[:, :], in_=wp_d)
                w2p_sb = consts.tile([32, 1024], BF16, tag="w2p")
                nc.sync.dma_start(out=w2p_sb[:, :], in_=w2p_d)
                ones_sb = consts.tile([1, 128], F32, tag="ones")
                nc.vector.memset(ones_sb[:, :], 1.0)
                # xwr [128, 4096]: cols 0:2048 layer-1 (rows 0:32/64:96 loaded
                # per-iteration), cols 2048:4096 layer-2 (l2 rows written by
                # the device restructure). All other rows must read as zero.
                xwr_sb = pool_xwr.tile([128, 4096], BF16, tag="xwr")
                nc.vector.memset(xwr_sb[:, :], 0.0)
                return (wbf_sb, wp_sb, w2p_sb, ones_sb, xwr_sb)

            def tail(state, h2B, y1bf, xwr_sb, oc2):
                (wbf_sb, wp_sb, w2p_sb, ones_sb, _xwr) = state
                # ---- contraction layer 2: 4 accumulate chains, one per PE
                # column tile; partials reduced by the sel4 matmul ----
                nc.tensor.matmul(out=oc2[0:32, :], lhsT=wp_sb[0:1, BIAS2],
                                 rhs=ones_sb[:, :],
                                 start=True, stop=False, skip_group_check=True)
                nc.tensor.matmul(out=oc2[32:64, :], lhsT=wbf_sb[0:32, ROOT2B],
                                 rhs=y1bf[:, :],
                                 start=True, stop=False, skip_group_check=True)
                for p in range(0 if "c2" in skip else 64):
                    q = 32 * (p % 4)
                    nc.tensor.matmul(
                        out=oc2[q:q + 32, :],
                        lhsT=xwr_sb[:, 2048 + 32 * p:2048 + 32 * p + 32],
                        rhs=h2B[:, 128 * p:128 * p + 128],
                        start=(2 <= p < 4), stop=(p >= 60),
                        skip_group_check=True, tile_position=(0, q),
                    )

                # reduce the 4 partial groups: evac + sel4 matmul
                oce2 = pool_misc.tile([128, 128], BF16, tag="oce2")
                nc.scalar.activation(out=oce2[:, :], in_=oc2[:, :],
                                     func=AF.Copy)
                o2f = pool_po.tile([C, 128], F32, tag="po", name="o2f")
                nc.tensor.matmul(out=o2f[:, :], lhsT=wbf_sb[0:128, SEL4],
                                 rhs=oce2[:, :], start=True, stop=True,
                                 skip_group_check=True)
                gv_sb = pool_misc.tile([C, 1], F32, tag="gv")
                h2f_sb = pool_misc.tile([C, 128], F32, tag="h2f")
                nc.scalar.activation(out=h2f_sb[:, :], in_=o2f[:, :],
                                     func=AF.Relu, accum_out=gv_sb[:, :])

                # ---- head: dense(64, relu)/128 -> dense(10) -> softmax ----
                d_ps = pool_po.tile([64, 1], F32, tag="po", name="d")
                nc.tensor.matmul(out=d_ps[:, :], lhsT=wp_sb[0:32, DW],
                                 rhs=gv_sb[:, :], start=True, stop=True)
                d_sb = pool_misc.tile([64, 1], F32, tag="d")
                nc.scalar.activation(out=d_sb[:, :], in_=d_ps[:, :], func=AF.Relu,
                                     bias=wp_sb[0:64, DB], scale=1.0 / 128.0)
                lg_ps = pool_po.tile([1, 10], F32, tag="po", name="lg")
                nc.tensor.matmul(out=lg_ps[:, :], lhsT=d_sb[:, :],
                                 rhs=wp_sb[0:64, OW], start=True, stop=True)
                lg_sb = pool_misc.tile([1, 10], F32, tag="lg")
                nc.vector.tensor_copy(lg_sb[:, :], lg_ps[:, :])
                nc.sync.dma_start(out=out_d, in_=lg_sb[:, :])

            def body(state, pipelined=False):
                (wbf_sb, wp_sb, w2p_sb, ones_sb, xwr_sb) = state
                # eA loads chunked (512 cols) so each h1 matmul only
                # waits for its own slice
                eA_sb = pool_eA.tile([S, E], BF16, tag="eA")
                if "eadma" not in skip:
                    for r in range(4):
                        nc.sync.dma_start(
                            out=eA_sb[:, 4096 * r:4096 * (r + 1)],
                            in_=eA_d[:, 4096 * r:4096 * (r + 1)])
                xc_sb = consts.tile([17, 128], F32, tag="xc")
                nc.sync.dma_start(out=xc_sb[:, :], in_=xc_d)

                nc.sync.dma_start(out=xwr_sb[:, 0:2048], in_=xwr1_d)

                h1s = pool_h1s.tile([128, E], BF16, tag="h1s")
                h2B = pool_h2B.tile([128, 8192], BF16, tag="h2B")
                y1bf = pool_misc.tile([C, 128], BF16, tag="y1bf")
                # contraction psums: 4 accumulate chains per layer, one per
                # PE column tile, reduced by the sel4 matmul at the end
                oc = pool_po.tile([128, 128], F32, tag="po", name="oc")
                oc2 = pool_po.tile([128, 128], F32, tag="po", name="oc2")
                if pipelined:
                    # previous iteration's tail: overlaps this iteration's
                    # input DMAs and MLP phase (loop-carried tile reads)
                    tail(state, h2B, y1bf, xwr_sb, oc2)

                nc.tensor.matmul(out=oc[0:32, :], lhsT=wp_sb[0:1, BIAS1],
                                 rhs=ones_sb[:, :],
                                 start=True, stop=False, skip_group_check=True)

                # ---- h1 (both layers fused, M=128, K=8) ----
                for r in range(0 if "h1" in skip else 16):
                    ps = pool_pmm.tile([128, 1024], F32, tag="pmm", name=f"h1_{r}")
                    for hb in range(2):
                        nc.tensor.matmul(
                            out=ps[:, 512 * hb:512 * (hb + 1)],
                            lhsT=wbf_sb[0:8, W1AB],
                            rhs=eA_sb[:, 1024 * r + 512 * hb:
                                      1024 * r + 512 * (hb + 1)],
                            start=True, stop=True,
                        )
                    dst = h1s[:, 1024 * r:1024 * (r + 1)]
                    if r % 2 == 1:
                        nc.vector.tensor_scalar_max(dst, ps[:, :], 0.0)
                    else:
                        nc.scalar.activation(out=dst, in_=ps[:, :], func=AF.Relu)

                # ---- h2 (both layers fused, M=64, K=128, j-pair packing) ----
                # group g: 16 edge-js (8 pairs). Host orders eA columns as
                # (g, j2, pp, i) so each (j2, bank) matmul streams a
                # CONTIGUOUS 512-col run of h1s (strided rhs is ~4x slower).
                for g in range(0 if "h2" in skip else 8):
                    ps = pool_pmm.tile([128, 1024], F32, tag="pmm", name=f"h2_{g}")
                    for j2 in range(2):
                        for hb in range(2):
                            lo = 2048 * g + 1024 * j2 + 512 * hb
                            nc.tensor.matmul(
                                out=ps[64 * j2:64 * j2 + 64,
                                       512 * hb:512 * (hb + 1)],
                                lhsT=wbf_sb[0:128, WB],
                                rhs=h1s[:, lo:lo + 512],
                                start=True, stop=True,
                                tile_position=(0, 64 * j2),
                            )
                    dst = h2B[:, 1024 * g:1024 * (g + 1)]
                    if g % 2 == 0:
                        nc.vector.tensor_scalar_max(dst, ps[:, :], 0.0)
                    else:
                        nc.scalar.activation(out=dst, in_=ps[:, :], func=AF.Relu)
                    if g == 0:
                        # group-1 chain seed (needs xc; placed here so the
                        # matmul sits in the busy stream, not at body top)
                        nc.tensor.matmul(
                            out=oc[32:64, :], lhsT=wp_sb[0:16, ROOT1],
                            rhs=xc_sb[0:16, :],
                            start=True, stop=False, skip_group_check=True)
                # clean single-mode phase: 4 accumulate chains across PE
                # column tiles (0,0)/(0,32)/(0,64)/(0,96)
                for p in range(0 if "c1" not in skip else 64, 64):
                    q = 32 * (p % 4)
                    nc.tensor.matmul(
                        out=oc[q:q + 32, :],
                        lhsT=xwr_sb[:, 32 * p:32 * p + 32],
                        rhs=h2B[:, 128 * p:128 * p + 128],
                        start=(2 <= p < 4), stop=(p >= 60),
                        skip_group_check=True, tile_position=(0, q),
                    )

                # reduce the 4 partial groups + leaky relu (mask is all-ones)
                oce = pool_misc.tile([128, 128], BF16, tag="oce")
                nc.scalar.activation(out=oce[:, :], in_=oc[:, :], func=AF.Copy)
                o1f = pool_po.tile([C, 128], F32, tag="po", name="o1f")
                nc.tensor.matmul(out=o1f[:, :], lhsT=wbf_sb[0:128, SEL4],
                                 rhs=oce[:, :], start=True, stop=True,
                                 skip_group_check=True)
                y1f = pool_misc.tile([C, 128], F32, tag="y1f")
                nc.scalar.activation(out=y1f[:, :], in_=o1f[:, :],
                                     func=AF.Copy)
                nc.vector.scalar_tensor_tensor(
                    out=y1bf[:, :], in0=y1f[:, :], scalar=0.05, in1=y1f[:, :],
                    op0=ALU.mult, op1=ALU.max)

                # ---- yW = y1 @ w2p, restructured into xwr cols 2048:4096 ----
                yw_ps = pool_pmm.tile([128, 1024], F32, tag="pmm", name="yw")
                for hb in range(2):
                    nc.tensor.matmul(out=yw_ps[:, 512 * hb:512 * (hb + 1)],
                                     lhsT=y1bf[:, :],
                                     rhs=w2p_sb[:, 512 * hb:512 * (hb + 1)],
                                     start=True, stop=True)
                yw_bf = pool_misc.tile([128, 1024], BF16, tag="ywbf")
                nc.scalar.activation(out=yw_bf[:, 0:512], in_=yw_ps[:, 0:512],
                                     func=AF.Copy)
                nc.vector.tensor_copy(yw_bf[:, 512:1024], yw_ps[:, 512:1024])
                ywd = dram_pool.tile([128, 1024], BF16, tag="ywd")
                ywr = ywd[:, :].rearrange("(p j2) (k c) -> j2 k p c", j2=2, c=32)
                nc.sync.dma_start(out=ywd[:, :], in_=yw_bf[:, :])
                for j2 in range(2):
                    nc.sync.dma_start(
                        out=xwr_sb[64 * j2 + 32:64 * j2 + 64, 2048:4096
                                   ].rearrange("k (p c) -> k p c", c=32),
                        in_=ywr[j2, :, :, :],
                    )

                if not pipelined:
                    tail(state, h2B, y1bf, xwr_sb, oc2)

            if loop_n is not None and loop_n > 1:
                if unroll:
                    st = setup()
                    for _ in range(loop_n):
                        body(st)
                elif hoist:
                    st = setup()
                    # manual unroll inside the hw loop: one all-engine
                    # back-edge barrier per unroll_factor iterations
                    u = unroll_factor
                    while u > 1 and loop_n % u:
                        u -= 1
                    with tc.For_i(0, loop_n // u, 1, hint_engines=(
                            mybir.EngineType.PE, mybir.EngineType.DVE,
                            mybir.EngineType.Activation, mybir.EngineType.SP)):
                        for _ in range(u):
                            body(st, pipelined=True)
                else:
                    with tc.For_i(0, loop_n, 1, hint_engines=(
                            mybir.EngineType.PE, mybir.EngineType.DVE,
                            mybir.EngineType.Activation, mybir.EngineType.SP)):
                        body(setup())
            else:
                body(setup())
    nc.compile()
    return nc


def prep_inputs(x, a, e, w1a, b1a, w1b, b1b, w1c, b1c, root1, bias1,
                w2a, b2a, w2b, b2b, w2c, b2c, root2, bias2, dw, db, ow, ob):
    """Host-side shard + layout prep. Returns in_maps (one per core)."""
    x = np.asarray(x, np.float32)
    a = np.asarray(a, np.float32)
    e = np.asarray(e, np.float32)
    # The device program folds a into e through the zero-bias ReLU MLP.
    for b_ in (b1a, b1b, b1c, b2a, b2b, b2c):
        assert np.abs(np.asarray(b_)).max() == 0.0, "nonzero MLP bias unsupported"
    # GraphMasking node-validity column is structurally all-ones here; the
    # device program relies on it (mask multiplies are elided).
    assert np.all(x[:, :, 16] == 1.0), "non-trivial node mask unsupported"

    import ml_dtypes
    bf16 = ml_dtypes.bfloat16

    wbf = np.zeros((128, WBF_COLS), bf16)
    wbf[0:8, 0:64] = np.asarray(w1a).astype(bf16)
    wbf[0:8, 64:128] = np.asarray(w2a).astype(bf16)
    wbf[0:64, 128:160] = np.asarray(w1b).astype(bf16)
    wbf[64:128, 160:192] = np.asarray(w2b).astype(bf16)
    wbf[0:32, ROOT2B] = np.asarray(root2).astype(bf16)
    sel4 = np.zeros((128, 32), np.float32)
    sel4[np.arange(128), np.arange(128) % 32] = 1.0
    wbf[:, SEL4] = sel4.astype(bf16)

    wp = np.zeros((128, WP_COLS), np.float32)
    wp[0:16, ROOT1] = np.asarray(root1)
    wp[0:32, DW] = np.asarray(dw)
    wp[0:64, OW] = np.asarray(ow)
    wp[0:64, DB] = np.asarray(db).reshape(64, 1)
    wp[0:1, BIAS1] = np.asarray(bias1).reshape(1, 32)
    wp[0:1, BIAS2] = np.asarray(bias2).reshape(1, 32)
    wp[0:1, OB] = np.asarray(ob).reshape(1, 10)

    # w1p[f, 32k+c] = w1c[k, f*32+c]; w2p[cin, 32k+c] = w2c[k, 32*cin+c]
    w1p = np.ascontiguousarray(
        np.asarray(w1c, np.float32).reshape(32, 16, 32)
        .transpose(1, 0, 2).reshape(16, 1024))
    w2p = np.ascontiguousarray(
        np.asarray(w2c).reshape(32, 32, 32).transpose(1, 0, 2).reshape(32, 1024)
    ).astype(bf16)

    in_maps = []
    for g in range(B):
        eA = np.ascontiguousarray(
            (e[g] * a[g][..., None]).transpose(2, 1, 0).reshape(S, E))
        # reorder edge columns j-major -> (group, j2=j%2, pp=(j%16)//2, i) so
        # the h2 matmuls stream contiguous runs
        eA = np.ascontiguousarray(
            eA.reshape(S, 8, 8, 2, 128).transpose(0, 1, 3, 2, 4).reshape(S, E)
        ).astype(bf16)
        x16 = x[g, :, 0:16]                     # [128, 16]
        xc = np.ascontiguousarray(x[g].T)       # [17, 128]
        # xwr1[32*j2 + k, 32*p + c] = xW[2p + j2, 32k + c]
        xW = (x16 @ w1p).astype(np.float32)     # [128, 1024]
        # merged layout [128, 2048]: rows 64*j2+0:32 = layer-1 blk(j2),
        # rows 64*j2+32:64 = zeros (layer-2 k slots read as 0 in cols 0:2048)
        xwr1 = np.zeros((128, 2048), np.float32)
        for j2 in range(2):
            blk = xW[j2::2].reshape(64, 32, 32).transpose(1, 0, 2).reshape(32, 2048)
            xwr1[64 * j2:64 * j2 + 32, :] = blk
        in_maps.append(dict(
            eA=eA, xwr1=xwr1.astype(bf16), xc=xc,
            wbf=wbf, wp=wp, w2p=w2p))
    return in_maps


_NC_CACHE = {}


def _get_nc(loop_n=None):
    key = loop_n
    if key not in _NC_CACHE:
        _NC_CACHE[key] = build_nc(loop_n)
    return _NC_CACHE[key]


def kernel(**inputs) -> np.ndarray:
    in_maps = prep_inputs(**inputs)
    nc = _get_nc()
    # The axon-tunneled device occasionally reports a transient
    # "exec unit unrecoverable" on the first dispatch after idle; retry.
    last = None
    for _ in range(3):
        try:
            res = run_bass_kernel_spmd(nc, in_maps, core_ids=list(range(B)))
            lg = np.concatenate(
                [res.results[g]["out"] for g in range(B)], axis=0)
            lg = lg.astype(np.float64) + np.asarray(inputs["ob"], np.float64)
            ex = np.exp(lg - lg.max(axis=1, keepdims=True))
            return (ex / ex.sum(axis=1, keepdims=True)).astype(np.float32)
        except Exception as ex:  # noqa: BLE001
            last = ex
    raise last



# revision 28
# speedup vs baseline: 1.8023x; 1.2512x over previous
"""Trainium2 Bass kernel for nn_Net_64982855188859 (ECC graph-conv net).

Network (per graph g of B=8, N=128 nodes, F=16 feats, S=8 edge feats, C=32):
  mask = x[..., -1:], h = x[..., :16]
  h = leaky_relu(ECC1(h, a, e) * mask)          ECC: per-edge MLP -> filters
  h = relu(ECC2(h, a, e)) * mask
  out = softmax(relu(mean_nodes(h) @ dw) @ ow)

Restructuring (all exact):
 1. w1c/w2c factor through the node features: xW[j,(k,c)] = sum_f x[j,f]
    w1c[k, f*C+c], so out[i,c] = sum_{j,k} a[i,j] h2[(j,i),k] xW[j,(k,c)].
 2. a >= 0 commutes through the zero-bias per-edge ReLU MLP, so the host
    pre-multiplies e by a (eA) and no masking happens on device.
 3. Both layers' per-edge MLPs depend only on eA, so they are FUSED:
    h1 = eA @ [w1a|w2a] (M=128), h2 = relu(h1) @ blockdiag(w1b,w2b)
    (M=64, K=128), halving PE streaming vs per-layer passes.
 4. h2 psum partition layout (j2, layer, k) via tile_position col packing:
    one [128, 8192] tensor h2B holds both layers' per-edge features in
    contraction-chunk layout (chunk = j-pair, K=128 with the other layer's
    rhs rows zeroed).
 5. Contraction is flipped: lhsT = h2B chunk [128,128] (FWL-fast weight
    loads), rhs = 32 cols of xwr -> out[i, c] in psum. Layer-1 xwr is
    HOST-computed (x is an input); layer-2 ywr needs one DRAM roundtrip.
 6. Per-core work = one graph (data-parallel over B=8 across 8 cores).
"""

import numpy as np

import concourse.bass as bass
import concourse.bacc as bacc
import concourse.mybir as mybir
import concourse.tile as tile
from concourse.bass_utils import run_bass_kernel_spmd

F32 = mybir.dt.float32
BF16 = mybir.dt.bfloat16
AF = mybir.ActivationFunctionType
ALU = mybir.AluOpType

B, N, F, S, C = 8, 128, 16, 8, 32
E = N * N               # 16384 edges per graph

# wbf column layout ([128, 256] bf16, shared across cores)
W1AB = slice(0, 128)      # rows 0:8  = [w1a | w2a]
WB = slice(128, 192)      # [128,64] blockdiag: rows 0:64 cols 0:32 = w1b,
                          #                     rows 64:128 cols 32:64 = w2b
ROOT2B = slice(192, 224)  # rows 0:32 = root2 (bf16)
SEL4 = slice(224, 256)    # sel4[p, c] = (p % 32 == c): 4-group psum reducer
WBF_COLS = 256
# wp column layout ([128, 192] fp32, shared)
ROOT1 = slice(0, 32)      # rows 0:16
DW = slice(32, 96)        # rows 0:32
OW = slice(96, 106)       # rows 0:64
DB = slice(106, 107)      # rows 0:64 (column vector)
BIAS1 = slice(112, 144)   # row 0
BIAS2 = slice(144, 176)   # row 0
OB = slice(176, 186)      # row 0
WP_COLS = 192


def build_nc(loop_n: int | None = None, unroll: bool = False,
             hoist: bool = True, skip: tuple = (), unroll_factor: int = 8):
    nc = bacc.Bacc("TRN2", target_bir_lowering=False, debug=False)
    eA_d = nc.dram_tensor("eA", [S, E], BF16, kind="ExternalInput").ap()
    xwr1_d = nc.dram_tensor("xwr1", [128, 2048], BF16, kind="ExternalInput").ap()
    xc_d = nc.dram_tensor("xc", [17, 128], F32, kind="ExternalInput").ap()
    wbf_d = nc.dram_tensor("wbf", [128, WBF_COLS], BF16, kind="ExternalInput").ap()
    wp_d = nc.dram_tensor("wp", [128, WP_COLS], F32, kind="ExternalInput").ap()
    w2p_d = nc.dram_tensor("w2p", [32, 1024], BF16, kind="ExternalInput").ap()
    out_d = nc.dram_tensor("out", [1, 10], F32, kind="ExternalOutput").ap()

    with tile.TileContext(nc) as tc:
        with (
            tc.tile_pool(name="consts", bufs=1) as consts,
            tc.tile_pool(name="eA", bufs=1) as pool_eA,
            tc.tile_pool(name="h1sp", bufs=2) as pool_h1s,
            tc.tile_pool(name="h2Bp", bufs=2) as pool_h2B,
            tc.tile_pool(name="xwr", bufs=1) as pool_xwr,
            tc.tile_pool(name="pmm", bufs=3, space="PSUM") as pool_pmm,
            tc.tile_pool(name="po", bufs=2, space="PSUM") as pool_po,
            tc.tile_pool(name="misc", bufs=1) as pool_misc,
            tc.tile_pool(name="dram", bufs=1, space="DRAM") as dram_pool,
        ):
            def setup():
                # ---- loop-invariant: weights + the xwr zero regions ----
                wbf_sb = consts.tile([128, WBF_COLS], BF16, tag="wbf")
                nc.sync.dma_start(out=wbf_sb[:, :], in_=wbf_d)
                wp_sb = consts.tile([128, WP_COLS], F32, tag="wp")
                nc.sync.dma_start(out=wp_sb[:, :], in_=wp_d)
                w2p_sb = consts.tile([32, 1024], BF16, tag="w2p")
                nc.sync.dma_start(out=w2p_sb[:, :], in_=w2p_d)
                ones_sb = consts.tile([1, 128], F32, tag="ones")
                nc.vector.memset(ones_sb[:, :], 1.0)
                # xwr [128, 4096]: cols 0:2048 layer-1 (rows 0:32/64:96 loaded
                # per-iteration), cols 2048:4096 layer-2 (l2 rows written by
                # the device restructure). All other rows must read as zero.
                xwr_sb = pool_xwr.tile([128, 4096], BF16, tag="xwr")
                nc.vector.memset(xwr_sb[:, :], 0.0)
                return (wbf_sb, wp_sb, w2p_sb, ones_sb, xwr_sb)

            def tail(state, h2B, y1bf, xwr_sb, oc2):
                (wbf_sb, wp_sb, w2p_sb, ones_sb, _xwr) = state
                # ---- contraction layer 2: 4 accumulate chains, one per PE
                # column tile; partials reduced by the sel4 matmul ----
                nc.tensor.matmul(out=oc2[0:32, :], lhsT=wp_sb[0:1, BIAS2],
                                 rhs=ones_sb[:, :],
                                 start=True, stop=False, skip_group_check=True)
                nc.tensor.matmul(out=oc2[32:64, :], lhsT=wbf_sb[0:32, ROOT2B],
                                 rhs=y1bf[:, :],
                                 start=True, stop=False, skip_group_check=True)
                for p in range(0 if "c2" in skip else 64):
                    q = 32 * (p % 4)
                    nc.tensor.matmul(
                        out=oc2[q:q + 32, :],
                        lhsT=xwr_sb[:, 2048 + 32 * p:2048 + 32 * p + 32],
                        rhs=h2B[:, 128 * p:128 * p + 128],
                        start=(2 <= p < 4), stop=(p >= 60),
                        skip_group_check=True, tile_position=(0, q),
                    )

                # reduce the 4 partial groups: evac + sel4 matmul
                oce2 = pool_misc.tile([128, 128], BF16, tag="oce2")
                nc.scalar.activation(out=oce2[:, :], in_=oc2[:, :],
                                     func=AF.Copy)
                o2f = pool_po.tile([C, 128], F32, tag="po", name="o2f")
                nc.tensor.matmul(out=o2f[:, :], lhsT=wbf_sb[0:128, SEL4],
                                 rhs=oce2[:, :], start=True, stop=True,
                                 skip_group_check=True)
                gv_sb = pool_misc.tile([C, 1], F32, tag="gv")
                h2f_sb = pool_misc.tile([C, 128], F32, tag="h2f")
                nc.scalar.activation(out=h2f_sb[:, :], in_=o2f[:, :],
                                     func=AF.Relu, accum_out=gv_sb[:, :])

                # ---- head: dense(64, relu)/128 -> dense(10) -> softmax ----
                d_ps = pool_po.tile([64, 1], F32, tag="po", name="d")
                nc.tensor.matmul(out=d_ps[:, :], lhsT=wp_sb[0:32, DW],
                                 rhs=gv_sb[:, :], start=True, stop=True)
                d_sb = pool_misc.tile([64, 1], F32, tag="d")
                nc.scalar.activation(out=d_sb[:, :], in_=d_ps[:, :], func=AF.Relu,
                                     bias=wp_sb[0:64, DB], scale=1.0 / 128.0)
                lg_ps = pool_po.tile([1, 10], F32, tag="po", name="lg")
                nc.tensor.matmul(out=lg_ps[:, :], lhsT=d_sb[:, :],
                                 rhs=wp_sb[0:64, OW], start=True, stop=True)
                lg_sb = pool_misc.tile([1, 10], F32, tag="lg")
                nc.vector.tensor_copy(lg_sb[:, :], lg_ps[:, :])
                nc.sync.dma_start(out=out_d, in_=lg_sb[:, :])

            def body(state, pipelined=False):
                (wbf_sb, wp_sb, w2p_sb, ones_sb, xwr_sb) = state
                # eA loads chunked (512 cols) so each h1 matmul only
                # waits for its own slice
                eA_sb = pool_eA.tile([S, E], BF16, tag="eA")
                if "eadma" not in skip:
                    for r in range(4):
                        nc.sync.dma_start(
                            out=eA_sb[:, 4096 * r:4096 * (r + 1)],
                            in_=eA_d[:, 4096 * r:4096 * (r + 1)])
                xc_sb = consts.tile([17, 128], F32, tag="xc")
                nc.sync.dma_start(out=xc_sb[:, :], in_=xc_d)

                nc.sync.dma_start(out=xwr_sb[:, 0:2048], in_=xwr1_d)

                h1s = pool_h1s.tile([128, E], BF16, tag="h1s")
                h2B = pool_h2B.tile([128, 8192], BF16, tag="h2B")
                y1bf = pool_misc.tile([C, 128], BF16, tag="y1bf")
                # contraction psums: 4 accumulate chains per layer, one per
                # PE column tile, reduced by the sel4 matmul at the end
                oc = pool_po.tile([128, 128], F32, tag="po", name="oc")
                oc2 = pool_po.tile([128, 128], F32, tag="po", name="oc2")
                if pipelined:
                    # previous iteration's tail: overlaps this iteration's
                    # input DMAs and MLP phase (loop-carried tile reads)
                    tail(state, h2B, y1bf, xwr_sb, oc2)

                nc.tensor.matmul(out=oc[0:32, :], lhsT=wp_sb[0:1, BIAS1],
                                 rhs=ones_sb[:, :],
                                 start=True, stop=False, skip_group_check=True)

                # ---- h1 (both layers fused, M=128, K=8) ----
                for r in range(0 if "h1" in skip else 16):
                    ps = pool_pmm.tile([128, 1024], F32, tag="pmm", name=f"h1_{r}")
                    for hb in range(2):
                        nc.tensor.matmul(
                            out=ps[:, 512 * hb:512 * (hb + 1)],
                            lhsT=wbf_sb[0:8, W1AB],
                            rhs=eA_sb[:, 1024 * r + 512 * hb:
                                      1024 * r + 512 * (hb + 1)],
                            start=True, stop=True,
                        )
                    dst = h1s[:, 1024 * r:1024 * (r + 1)]
                    if r % 2 == 1:
                        nc.vector.tensor_scalar_max(dst, ps[:, :], 0.0)
                    else:
                        nc.scalar.activation(out=dst, in_=ps[:, :], func=AF.Relu)

                # ---- h2 (both layers fused, M=64, K=128, j-pair packing) ----
                # group g: 16 edge-js (8 pairs). Host orders eA columns as
                # (g, j2, pp, i) so each (j2, bank) matmul streams a
                # CONTIGUOUS 512-col run of h1s (strided rhs is ~4x slower).
                for g in range(0 if "h2" in skip else 8):
                    ps = pool_pmm.tile([128, 1024], F32, tag="pmm", name=f"h2_{g}")
                    for j2 in range(2):
                        for hb in range(2):
                            lo = 2048 * g + 1024 * j2 + 512 * hb
                            nc.tensor.matmul(
                                out=ps[64 * j2:64 * j2 + 64,
                                       512 * hb:512 * (hb + 1)],
                                lhsT=wbf_sb[0:128, WB],
                                rhs=h1s[:, lo:lo + 512],
                                start=True, stop=True,
                                tile_position=(0, 64 * j2),
                            )
                    dst = h2B[:, 1024 * g:1024 * (g + 1)]
                    if g % 2 == 0:
                        nc.vector.tensor_scalar_max(dst, ps[:, :], 0.0)
                    else:
                        nc.scalar.activation(out=dst, in_=ps[:, :], func=AF.Relu)
                    if g == 0:
                        # group-1 chain seed (needs xc; placed here so the
                        # matmul sits in the busy stream, not at body top)
                        nc.tensor.matmul(
                            out=oc[32:64, :], lhsT=wp_sb[0:16, ROOT1],
                            rhs=xc_sb[0:16, :],
                            start=True, stop=False, skip_group_check=True)
                # clean single-mode phase: 4 accumulate chains across PE
                # column tiles (0,0)/(0,32)/(0,64)/(0,96)
                for p in range(0 if "c1" not in skip else 64, 64):
                    q = 32 * (p % 4)
                    nc.tensor.matmul(
                        out=oc[q:q + 32, :],
                        lhsT=xwr_sb[:, 32 * p:32 * p + 32],
                        rhs=h2B[:, 128 * p:128 * p + 128],
                        start=(2 <= p < 4), stop=(p >= 60),
                        skip_group_check=True, tile_position=(0, q),
                    )

                # reduce the 4 partial groups + leaky relu (mask is all-ones)
                oce = pool_misc.tile([128, 128], BF16, tag="oce")
                nc.scalar.activation(out=oce[:, :], in_=oc[:, :], func=AF.Copy)
                o1f = pool_po.tile([C, 128], F32, tag="po", name="o1f")
                nc.tensor.matmul(out=o1f[:, :], lhsT=wbf_sb[0:128, SEL4],
                                 rhs=oce[:, :], start=True, stop=True,
                                 skip_group_check=True)
                y1f = pool_misc.tile([C, 128], F32, tag="y1f")
                nc.scalar.activation(out=y1f[:, :], in_=o1f[:, :],
                                     func=AF.Copy)
                nc.vector.scalar_tensor_tensor(
                    out=y1bf[:, :], in0=y1f[:, :], scalar=0.05, in1=y1f[:, :],
                    op0=ALU.mult, op1=ALU.max)

                # ---- yW = y1 @ w2p, restructured into xwr cols 2048:4096 ----
                yw_ps = pool_pmm.tile([128, 1024], F32, tag="pmm", name="yw")
                for hb in range(2):
                    nc.tensor.matmul(out=yw_ps[:, 512 * hb:512 * (hb + 1)],
                                     lhsT=y1bf[:, :],
                                     rhs=w2p_sb[:, 512 * hb:512 * (hb + 1)],
                                     start=True, stop=True)
                yw_bf = pool_misc.tile([128, 1024], BF16, tag="ywbf")
                nc.scalar.activation(out=yw_bf[:, 0:512], in_=yw_ps[:, 0:512],
                                     func=AF.Copy)
                nc.vector.tensor_copy(yw_bf[:, 512:1024], yw_ps[:, 512:1024])
                ywd = dram_pool.tile([128, 1024], BF16, tag="ywd")
                ywr = ywd[:, :].rearrange("(p j2) (k c) -> j2 k p c", j2=2, c=32)
                nc.sync.dma_start(out=ywd[:, :], in_=yw_bf[:, :])
                for j2 in range(2):
                    nc.sync.dma_start(
                        out=xwr_sb[64 * j2 + 32:64 * j2 + 64, 2048:4096
                                   ].rearrange("k (p c) -> k p c", c=32),
                        in_=ywr[j2, :, :, :],
                    )

                if not pipelined:
                    tail(state, h2B, y1bf, xwr_sb, oc2)

            if loop_n is not None and loop_n > 1:
                if unroll:
                    st = setup()
                    for _ in range(loop_n):
                        body(st)
                elif hoist:
                    st = setup()
                    # manual unroll inside the hw loop: one all-engine
                    # back-edge barrier per unroll_factor iterations
                    u = unroll_factor
                    while u > 1 and loop_n % u:
                        u -= 1
                    with tc.For_i(0, loop_n // u, 1, hint_engines=(
                            mybir.EngineType.PE, mybir.EngineType.DVE,
                            mybir.EngineType.Activation, mybir.EngineType.SP)):
                        for _ in range(u):
                            body(st, pipelined=True)
                else:
                    with tc.For_i(0, loop_n, 1, hint_engines=(
                            mybir.EngineType.PE, mybir.EngineType.DVE,
                            mybir.EngineType.Activation, mybir.EngineType.SP)):
                        body(setup())
            else:
                body(setup())
    nc.compile()
    return nc


def prep_inputs(x, a, e, w1a, b1a, w1b, b1b, w1c, b1c, root1, bias1,
                w2a, b2a, w2b, b2b, w2c, b2c, root2, bias2, dw, db, ow, ob):
    """Host-side shard + layout prep. Returns in_maps (one per core)."""
    x = np.asarray(x, np.float32)
    a = np.asarray(a, np.float32)
    e = np.asarray(e, np.float32)
    # The device program folds a into e through the zero-bias ReLU MLP.
    for b_ in (b1a, b1b, b1c, b2a, b2b, b2c):
        assert np.abs(np.asarray(b_)).max() == 0.0, "nonzero MLP bias unsupported"
    # GraphMasking node-validity column is structurally all-ones here; the
    # device program relies on it (mask multiplies are elided).
    assert np.all(x[:, :, 16] == 1.0), "non-trivial node mask unsupported"

    import ml_dtypes
    bf16 = ml_dtypes.bfloat16

    wbf = np.zeros((128, WBF_COLS), bf16)
    wbf[0:8, 0:64] = np.asarray(w1a).astype(bf16)
    wbf[0:8, 64:128] = np.asarray(w2a).astype(bf16)
    wbf[0:64, 128:160] = np.asarray(w1b).astype(bf16)
    wbf[64:128, 160:192] = np.asarray(w2b).astype(bf16)
    wbf[0:32, ROOT2B] = np.asarray(root2).astype(bf16)
    sel4 = np.zeros((128, 32), np.float32)
    sel4[np.arange(128), np.arange(128) % 32] = 1.0
    wbf[:, SEL4] = sel4.astype(bf16)

    wp = np.zeros((128, WP_COLS), np.float32)
    wp[0:16, ROOT1] = np.asarray(root1)
    wp[0:32, DW] = np.asarray(dw)
    wp[0:64, OW] = np.asarray(ow)
    wp[0:64, DB] = np.asarray(db).reshape(64, 1)
    wp[0:1, BIAS1] = np.asarray(bias1).reshape(1, 32)
    wp[0:1, BIAS2] = np.asarray(bias2).reshape(1, 32)
    wp[0:1, OB] = np.asarray(ob).reshape(1, 10)

    # w1p[f, 32k+c] = w1c[k, f*32+c]; w2p[cin, 32k+c] = w2c[k, 32*cin+c]
    w1p = np.ascontiguousarray(
        np.asarray(w1c, np.float32).reshape(32, 16, 32)
        .transpose(1, 0, 2).reshape(16, 1024))
    w2p = np.ascontiguousarray(
        np.asarray(w2c).reshape(32, 32, 32).transpose(1, 0, 2).reshape(32, 1024)
    ).astype(bf16)

    in_maps = []
    for g in range(B):
        eA = np.ascontiguousarray(
            (e[g] * a[g][..., None]).transpose(2, 1, 0).reshape(S, E))
        # reorder edge columns j-major -> (group, j2=j%2, pp=(j%16)//2, i) so
        # the h2 matmuls stream contiguous runs
        eA = np.ascontiguousarray(
            eA.reshape(S, 8, 8, 2, 128).transpose(0, 1, 3, 2, 4).reshape(S, E)
        ).astype(bf16)
        x16 = x[g, :, 0:16]                     # [128, 16]
        xc = np.ascontiguousarray(x[g].T)       # [17, 128]
        # xwr1[32*j2 + k, 32*p + c] = xW[2p + j2, 32k + c]
        xW = (x16 @ w1p).astype(np.float32)     # [128, 1024]
        # merged layout [128, 2048]: rows 64*j2+0:32 = layer-1 blk(j2),
        # rows 64*j2+32:64 = zeros (layer-2 k slots read as 0 in cols 0:2048)
        xwr1 = np.zeros((128, 2048), np.float32)
        for j2 in range(2):
            blk = xW[j2::2].reshape(64, 32, 32).transpose(1, 0, 2).reshape(32, 2048)
            xwr1[64 * j2:64 * j2 + 32, :] = blk
        in_maps.append(dict(
            eA=eA, xwr1=xwr1.astype(bf16), xc=xc,
            wbf=wbf, wp=wp, w2p=w2p))
    return in_maps


_NC_CACHE = {}


def _get_nc(loop_n=None):
    key = loop_n
    if key not in _NC_CACHE:
        _NC_CACHE[key] = build_nc(loop_n)
    return _NC_CACHE[key]


def kernel(**inputs) -> np.ndarray:
    in_maps = prep_inputs(**inputs)
    nc = _get_nc()
    # The axon-tunneled device occasionally reports a transient
    # "exec unit unrecoverable" on the first dispatch after idle; retry.
    last = None
    for _ in range(3):
        try:
            res = run_bass_kernel_spmd(nc, in_maps, core_ids=list(range(B)))
            lg = np.concatenate(
                [res.results[g]["out"] for g in range(B)], axis=0)
            lg = lg.astype(np.float64) + np.asarray(inputs["ob"], np.float64)
            ex = np.exp(lg - lg.max(axis=1, keepdims=True))
            return (ex / ex.sum(axis=1, keepdims=True)).astype(np.float32)
        except Exception as ex:  # noqa: BLE001
            last = ex
    raise last

